# revision 18
# baseline (speedup 1.0000x reference)
"""Trainium2 Bass kernel for nn_DifferentiableFeatureExtractor.

Strategy (8 NeuronCores, shard T=1048576 along time):
  - per-core extended domain EXT = S + 2048 halo = 133120 = 128 partitions x 1040
  - each partition holds a contiguous 1040-bar chunk plus a 256-bar AP halo
    (tile [128, 1296]); host supplies a 256-bar lead-in so partition 0's halo
    is real data (clamp-padded at the global left edge like the reference)
  - 20 EMAs as untruncated fp32 recurrences y = a*scale*s with s from
    tensor_tensor_scan (2-pass blocked scan; partition carry via a tiny PE
    shift-matmul read at col W-9 so scan2 covers [248, W) and no separate
    halo fill is needed).  The truncation correction (= (1-a)^K ~ 1e-3
    relative, smooth) is dropped - far inside the 2e-2 tolerance.
  - work is split across the three elementwise engines: DVE (vector),
    Pool (gpsimd; binary ops emulated via scalar_tensor_tensor for the
    0.6-efficiency path; never touches PSUM), and ACT (scalar) for all
    unary scale/abs/sqrt/copy work.
  - sliding max/min via log-doubling with shifted APs (halo-local)
  - rolling std via anchored windowed sums (doubling)
  - BARSLAST/MA_DYNAMIC as segmented scans (reset at cross events) with
    affine partition-carry fix-up (Aprefix trick: corrected = local +
    prefix-indicator * per-partition-initial); exact whenever the previous
    cross lies within the 2048-bar halo (diag output flags violations ->
    host fallback)
All heavy compute runs on device; host only shards, gathers, patches the
17 reference partial-window std bars, and handles the (never-observed)
diag fallback.
"""
import math

import numpy as np

import concourse.bacc as bacc
from concourse.bass_types import AP as BassAP
import concourse.mybir as mybir
from concourse import tile as ctile
from concourse.bass_utils import run_bass_kernel_spmd

F32 = mybir.dt.float32
Alu = mybir.AluOpType
Act = mybir.ActivationFunctionType

T = 1048576
NCORES = 8
S = T // NCORES            # 131072
P = 128
CH = 1040                  # chunk cols per partition
HP = 256                   # per-partition halo cols
W = HP + CH                # 1296
EXT = P * CH               # 133120
HALO = EXT - S             # 2048
DLEN = HP + EXT            # 133376
C0 = HP                    # chunk start col
LO = 248                   # extended-chunk start (8-col margin for lags)
NROWS = 30

# static truncation lengths (depend only on the reference's constant ALPHAS)
KS = [72, 72, 72, 286, 286, 286, 559, 89, 54, 47, 40, 30, 130, 30,
      30, 30, 30, 37, 37, 37]


class KB:
    """kernel builder with a simple big-tile freelist"""

    def __init__(self, alphas, anchor):
        self.alphas = [float(a) for a in alphas]
        self.anchor = float(anchor)
        nc = bacc.Bacc(None, target_bir_lowering=False)
        self.nc = nc
        self.DC = nc.dram_tensor("DC", [DLEN], F32, kind="ExternalInput")
        self.DH = nc.dram_tensor("DH", [DLEN], F32, kind="ExternalInput")
        self.DL = nc.dram_tensor("DL", [DLEN], F32, kind="ExternalInput")
        self.OUT = nc.dram_tensor("OUT", [NROWS * EXT], F32, kind="ExternalOutput")
        self.DIAG = nc.dram_tensor("DIAG", [2], F32, kind="ExternalOutput")
        self.free_big = []
        self.n_big = 0
        self.free_small = []
        self.n_small = 0
        self.free_row = []
        self.n_row = 0

    # ---- tile management ----
    def big(self):
        if self.free_big:
            return self.free_big.pop(0)
        t = self.pool.tile([P, W], F32, tag=f"big{self.n_big}")
        self.n_big += 1
        return t

    def rel(self, *ts):
        for t in ts:
            self.free_big.append(t)

    def small(self):
        if self.free_small:
            return self.free_small.pop()
        t = self.spool.tile([P, 1], F32, tag=f"small{self.n_small}")
        self.n_small += 1
        return t

    def rels(self, *ts):
        for t in ts:
            self.free_small.append(t)

    def row(self):
        if self.free_row:
            return self.free_row.pop()
        t = self.spool.tile([1, P + 1], F32, tag=f"row{self.n_row}")
        self.n_row += 1
        return t

    def relr(self, *ts):
        for t in ts:
            self.free_row.append(t)

    # ---- engine helpers ----
    def tt(self, eng, out, a, b, op):
        """binary tensor op (Pool supports only add/subtract/mult)."""
        if eng is self.G:
            assert op in (Alu.add, Alu.subtract, Alu.mult), op
        eng.tensor_tensor(out=out, in0=a, in1=b, op=op)

    # ---- IO ----
    def load_series(self, dram, eng="sync"):
        nc = self.nc
        t = self.big()
        base = dram[0:DLEN].rearrange("(a b) -> a b", a=1, b=DLEN)
        src_ap = BassAP(base.tensor, 0, [[CH, P], [1, W]])
        getattr(nc, eng).dma_start(out=t[:, 0:W], in_=src_ap)
        return t

    def store_row(self, r, t):
        nc = self.nc
        nc.sync.dma_start(
            out=self.OUT[r * EXT : (r + 1) * EXT].rearrange(
                "(p w) -> p w", p=P, w=CH
            ),
            in_=t[:, C0:W],
        )

    # ---- building blocks ----
    def ema(self, xt, i, scale=1.0, raw=False, xlo=0, carry_lo=None):
        """EMA of xt -> new tile.
        carry_lo=None: ONE scan over [xlo, W) warming up through the halo
          (requires (1-a)^HP tiny AND xt valid on [xlo, W)).
        carry_lo=lo: 2-pass blocked scan with PE carry; output valid [lo, W)
          (for slow alphas or inputs only valid from ~lo).
        Output y = a*scale*s via ACT unless raw."""
        nc = self.nc
        a = self.alphas[i]
        K = KS[i]
        c = 1.0 - a
        s = self.big()
        if carry_lo is None:
            assert c ** HP < 1e-5
            cbc = self.CONSTS[:, i : i + 1].broadcast_to([P, W - xlo])
            self.V.tensor_tensor_scan(
                out=s[:, xlo:W], data0=cbc,
                data1=xt[:, xlo:W], initial=0.0, op0=Alu.mult, op1=Alu.add,
            )
            lo = xlo
        else:
            lo = carry_lo
            V = min(CH, 2 * K + 16)
            cbc = self.CONSTS[:, i : i + 1].broadcast_to([P, V])
            self.V.tensor_tensor_scan(
                out=s[:, W - V : W], data0=cbc,
                data1=xt[:, W - V : W], initial=0.0, op0=Alu.mult, op1=Alu.add,
            )
            # carry[p] = s[p-1, ccol] (+ c^CH * s[p-2, ccol] for slow alphas)
            # enters one bar before col lo; chain step is CH bars either way.
            ccol = W - (C0 - lo) - 1
            mmat = self.M2.get(i, self.Sh1)
            pcar = self.pscol.tile([P, 1], F32, tag="pscol")
            self.mm(pcar[:, 0:1], mmat[:, :], s[:, ccol : ccol + 1])
            cbc2 = self.CONSTS[:, i : i + 1].broadcast_to([P, W - lo])
            self.V.tensor_tensor_scan(
                out=s[:, lo:W], data0=cbc2,
                data1=xt[:, lo:W], initial=pcar[:, 0:1], op0=Alu.mult, op1=Alu.add,
            )
        if raw:
            return s
        y = self.big()
        self.A.mul(y[:, lo:W], s[:, lo:W], a * scale)
        self.rel(s)
        return y

    def ema_corr(self, xt, i, scale=1.0):
        """EMA with the truncation correction (TEMA chains, i in 0..5):
        y = a*scale*(s[t] - c^K s[t-K]) on [0, W); head cols [0, K) read
        s[t-K] from the previous partition's tail via a PE shift (partition
        0 gets zero-history - it lies in the discarded global halo)."""
        nc = self.nc
        a = self.alphas[i]
        K = KS[i]
        c = 1.0 - a
        cK = float(c) ** K
        if c ** HP < 1e-5:
            s = self.ema(xt, i, raw=True)
        else:
            s = self.ema(xt, i, raw=True, carry_lo=0)
        q = self.big()
        self.V.scalar_tensor_tensor(
            out=q[:, K:W], in0=s[:, 0 : W - K], scalar=cK,
            in1=s[:, K:W], op0=Alu.mult, op1=Alu.subtract,
        )
        ph = self.pshalo.tile([P, 286], F32, tag="psh")
        self.mm(ph[:, 0:K], self.Sh1[:, :], s[:, CH - K : CH])
        self.V.scalar_tensor_tensor(
            out=q[:, 0:K], in0=ph[:, 0:K], scalar=cK,
            in1=s[:, 0:K], op0=Alu.mult, op1=Alu.subtract,
        )
        self.rel(s)
        y = self.big()
        self.A.mul(y[:, 0:W], q[:, 0:W], -a * scale)
        self.rel(q)
        return y

    def winchain(self, xt, jmax, op, eng):
        """doubling chain for sliding max/min, valid on ALL of [0, W) with
        partial windows at the left edge (dst[:, :sh] copied from cur)."""
        chain = {1: xt}
        cur = xt
        for j in range(jmax):
            sh = 1 << j
            dst = self.big()
            self.tt(
                eng, dst[:, sh:W], cur[:, sh:W], cur[:, 0 : W - sh], op,
            )
            self.G.tensor_copy(dst[:, 0:sh], cur[:, 0:sh])
            cur = dst
            chain[sh * 2] = dst
        return chain

    def wincombine(self, chain, n, op, eng):
        """window-n result from a doubling chain (partial windows left of
        col n-1, matching the reference's edge behaviour inside the halo)."""
        J = 1 << int(math.floor(math.log2(n)))
        r = n - J
        cur = chain[J]
        out = self.big()
        if r > 0:
            self.tt(
                eng, out[:, r:W], cur[:, r:W], cur[:, 0 : W - r], op,
            )
            self.G.tensor_copy(out[:, 0:r], cur[:, 0:r])
        else:
            eng.tensor_copy(out[:, 0:W], cur[:, 0:W])
        return out

    def winsum18(self, xt, xlo, eng):
        """rolling 18-window sum of xt (valid from xlo); returns tile valid
        [xlo+31, W)."""
        tiles = []
        cur = xt
        curlo = xlo
        chain1 = None
        for j in range(4):
            sh = 1 << j
            dst = self.big()
            tiles.append(dst)
            self.tt(
                eng, dst[:, curlo + sh : W], cur[:, curlo + sh : W],
                cur[:, curlo : W - sh], Alu.add,
            )
            cur = dst
            curlo += sh
            if j == 0:
                chain1 = dst  # window-2 sums
        out = self.big()
        # S18[t] = W16[t] + W2[t-16]
        lo = curlo + 16
        self.tt(eng, out[:, lo:W], cur[:, lo:W], chain1[:, lo - 16 : W - 16], Alu.add)
        for t in tiles:
            self.rel(t)
        return out, lo

    # ---- full pipeline ----
    def build(self):
        nc = self.nc
        with ctile.TileContext(nc) as tc:
            with tc.tile_pool(name="big", bufs=1) as pool, tc.tile_pool(
                name="small", bufs=1
            ) as spool, tc.tile_pool(name="psc", bufs=2, space="PSUM") as pscol, \
                 tc.tile_pool(name="psh", bufs=2, space="PSUM") as pshalo, \
                 tc.tile_pool(name="psr", bufs=2, space="PSUM") as psrow:
                self.pool = pool
                self.spool = spool
                self.pscol = pscol
                self.pshalo = pshalo
                self.psrow = psrow
                self.emit()
        nc.finalize()
        return nc

    def mm(self, out, lhsT, rhs):
        self.nc.tensor.matmul(out, lhsT, rhs, start=True, stop=True)

    def emit(self):
        nc = self.nc
        I32 = mybir.dt.int32
        self.V = nc.vector
        self.G = nc.gpsimd
        self.A = nc.scalar
        V_, G_, A_ = self.V, self.G, self.A

        self.one1 = self.spool.tile([P, 1], F32, tag="c_one1")
        nc.gpsimd.memset(self.one1[:, :], 1.0)
        self.nanch = self.spool.tile([P, 1], F32, tag="c_nanch")
        nc.gpsimd.memset(self.nanch[:, :], -self.anchor)
        # PE helper constants: ii[p, m] = m - p; Sh1 = (ii==1); Ident = (ii==0)
        ii = self.spool.tile([P, P], I32, tag="c_iota")
        nc.gpsimd.iota(ii[:, :], pattern=[[1, P]], base=0, channel_multiplier=-1)
        self.Sh1 = self.spool.tile([P, P], F32, tag="c_sh1")
        nc.vector.tensor_single_scalar(
            out=self.Sh1[:, :], in_=ii[:, :], scalar=1, op=Alu.is_equal
        )
        self.Ident = self.spool.tile([P, P], F32, tag="c_ident")
        nc.vector.tensor_single_scalar(
            out=self.Ident[:, :], in_=ii[:, :], scalar=0, op=Alu.is_equal
        )
        # second-order carry matrices for slow alphas (cF > 1e-10)
        self.M2 = {}
        sh2 = None
        for i, a in enumerate(self.alphas):
            cF = (1.0 - a) ** CH
            if cF > 1e-10:
                if sh2 is None:
                    sh2 = self.spool.tile([P, P], F32, tag="c_sh2")
                    nc.vector.tensor_single_scalar(
                        out=sh2[:, :], in_=ii[:, :], scalar=2, op=Alu.is_equal
                    )
                m = self.spool.tile([P, P], F32, tag=f"c_m2_{i}")
                nc.vector.scalar_tensor_tensor(
                    out=m[:, :], in0=sh2[:, :], scalar=float(cF), in1=self.Sh1[:, :],
                    op0=Alu.mult, op1=Alu.add,
                )
                self.M2[i] = m
        # per-conv scan multiplier constants: CONSTS[:, i] = 1 - alpha_i
        self.CONSTS = self.spool.tile([P, len(self.alphas)], F32, tag="c_scanmul")
        for i, a in enumerate(self.alphas):
            nc.gpsimd.memset(self.CONSTS[:, i : i + 1], 1.0 - a)
        self.ones11 = self.spool.tile([1, 1], F32, tag="c_ones11")
        nc.gpsimd.memset(self.ones11[:, :], 1.0)
        # TG: global ext index per bar (for `seen`); TGL: local index + 1
        self.TG = self.spool.tile([P, CH], I32, tag="c_tg")
        nc.gpsimd.iota(self.TG[:, :], pattern=[[1, CH]], base=0, channel_multiplier=CH)
        self.TGL = self.spool.tile([P, CH], I32, tag="c_tgl")
        nc.gpsimd.iota(self.TGL[:, :], pattern=[[1, CH]], base=1, channel_multiplier=0)

        Ht = self.load_series(self.DH)
        Lt = self.load_series(self.DL, eng="gpsimd")
        Ct = self.load_series(self.DC)

        # --- sliding-window chains: H on DVE, L on Pool ---
        hchain = self.winchain(Ht, 7, Alu.max, V_)
        lchain = self.winchain(Lt, 7, Alu.min, V_)

        def rsvf(nw):
            """rsv in [0,1] on [0, W) (left edge = partial windows)."""
            hh = self.wincombine(hchain, nw, Alu.max, V_)
            ll = self.wincombine(lchain, nw, Alu.min, V_)
            hl = self.big()
            self.tt(G_, hl[:, 0:W], hh[:, 0:W], ll[:, 0:W], Alu.subtract)
            rcp = self.big()
            V_.reciprocal_approx_fast(out=rcp[:, 0:W], in_=hl[:, 0:W])
            num = self.big()
            self.tt(G_, num[:, 0:W], Ct[:, 0:W], ll[:, 0:W], Alu.subtract)
            r0 = self.big()
            self.tt(G_, r0[:, 0:W], num[:, 0:W], rcp[:, 0:W], Alu.mult)
            self.rel(hh, ll, hl, rcp, num)
            rsv01 = self.big()
            V_.tensor_scalar(
                out=rsv01[:, 0:W], in0=r0[:, 0:W], scalar1=0.0, scalar2=1.0,
                op0=Alu.max, op1=Alu.min,
            )
            self.rel(r0)
            return rsv01

        def jof(Kv, Dv, rows, lo=0):
            k3 = self.big()
            A_.mul(k3[:, lo:W], Kv[:, lo:W], 3.0)
            d2 = self.big()
            A_.mul(d2[:, lo:W], Dv[:, lo:W], 2.0)
            Jv = self.big()
            self.tt(G_, Jv[:, lo:W], k3[:, lo:W], d2[:, lo:W], Alu.subtract)
            self.rel(k3, d2)
            for ridx, tt_ in zip(rows, (Kv, Dv, Jv)):
                if ridx is not None:
                    self.store_row(ridx, tt_)
            return Jv

        rsv1 = rsvf(204)
        rsv2 = rsvf(18)
        K1 = self.ema(rsv1, 6, 100.0, carry_lo=LO)
        K2 = self.ema(rsv2, 8, 100.0)
        self.rel(rsv1, rsv2)
        D1 = self.ema(K1, 7, carry_lo=LO)
        D2 = self.ema(K2, 9)
        J1 = jof(K1, D1, (9, 10, 11), lo=LO)
        J2 = jof(K2, D2, (12, 13, 14))
        self.rel(K1, D1, K2, D2)

        # --- TEMA3 + TEMAP2 chains (engine-split pairs) ---
        # EMA1/EMA2 carry a folded 3x; EMA3 unfolds it (ema is linear), so
        # TEMA3 = EMA1x3 - EMA2x3 + EMA3 with Pool-only combines.  The TEMA
        # chains use truncation-corrected EMAs (the T-ratio rows are the
        # precision-critical consumers).
        EMA1 = self.ema_corr(Ct, 0, 3.0)
        E21 = self.ema_corr(Ct, 3, 3.0)
        EMA2 = self.ema_corr(EMA1, 1)
        E221 = self.ema_corr(E21, 4)
        EMA3 = self.ema_corr(EMA2, 2, 1.0 / 3.0)
        E231 = self.ema_corr(E221, 5, 1.0 / 3.0)
        TEMA3 = self.big()
        d = self.big()
        self.tt(G_, d[:, 0:W], EMA1[:, 0:W], EMA2[:, 0:W], Alu.subtract)
        self.tt(G_, TEMA3[:, 0:W], d[:, 0:W], EMA3[:, 0:W], Alu.add)
        self.rel(EMA1, EMA2, EMA3, d)
        self.store_row(4, TEMA3)
        TEMAP2 = self.big()
        d = self.big()
        self.tt(G_, d[:, 0:W], E21[:, 0:W], E221[:, 0:W], Alu.subtract)
        self.tt(G_, TEMAP2[:, 0:W], d[:, 0:W], E231[:, 0:W], Alu.add)
        self.rel(E21, E221, E231, d)

        # --- stdp(C,18) anchored at global C[0] ---
        dev = self.big()
        self.A.activation(dev[:, 0:W], Ct[:, 0:W], Act.Identity, bias=self.nanch[:, 0:1])
        dev2 = self.big()
        self.A.activation(dev2[:, 0:W], Ct[:, 0:W], Act.Square, bias=self.nanch[:, 0:1])
        S18, lo1 = self.winsum18(dev, 0, V_)
        Q18, lo2 = self.winsum18(dev2, 0, G_)
        self.rel(dev, dev2)
        m = self.big()
        self.A.mul(m[:, lo1:W], S18[:, lo1:W], 1.0 / 18.0)
        ex2 = self.big()
        self.A.mul(ex2[:, lo2:W], Q18[:, lo2:W], 1.0 / 18.0)
        self.rel(S18, Q18)
        mm_ = self.big()
        self.A.square(mm_[:, lo1:W], m[:, lo1:W])
        var = self.big()
        self.tt(G_, var[:, lo1:W], ex2[:, lo1:W], mm_[:, lo1:W], Alu.subtract)
        G_.tensor_scalar_max(var[:, lo1:W], var[:, lo1:W], 0.0)
        DIS = self.big()
        self.A.activation(DIS[:, lo1:W], var[:, lo1:W], Act.Sqrt)
        self.rel(m, ex2, mm_, var)
        TEU3 = self.big()
        self.tt(G_, TEU3[:, C0:W], TEMA3[:, C0:W], DIS[:, C0:W], Alu.add)
        TED = self.big()
        self.tt(G_, TED[:, C0:W], TEMA3[:, C0:W], DIS[:, C0:W], Alu.subtract)
        self.store_row(3, TEU3)
        self.store_row(5, TED)
        self.rel(DIS, TEU3, TED)

        rsv3 = rsvf(9)
        rsvn = rsvf(36)
        K3 = self.ema(rsv3, 10, 100.0)
        KN3 = self.ema(rsvn, 12, 100.0)
        self.rel(rsv3, rsvn)
        D3 = self.ema(K3, 11)
        DN3 = self.ema(KN3, 13)
        J3 = jof(K3, D3, (15, 16, 17))
        JN3 = jof(KN3, DN3, (None, None, 18))
        self.rel(K3, D3, KN3, DN3, JN3)
        for ch_ in (hchain, lchain):
            for kk, tt_ in ch_.items():
                if kk > 1:
                    self.rel(tt_)
        self.rel(Ht, Lt)

        # --- T ratios (no +eps: |r| ~ 100 so the 1e-8 is immaterial).
        # Computed from col `lag` so JXb/F1/F2 are near-full-width and the
        # JX EMAs can warm up through the halo with a single scan.
        def tdiff(xt, lag, row_idx):
            lo = lag
            dt_ = self.big()
            self.tt(
                G_, dt_[:, lo:W], xt[:, lo:W], xt[:, 0 : W - lag],
                Alu.subtract,
            )
            ab = self.big()
            self.A.activation(ab[:, lo:W], xt[:, 0 : W - lag], Act.Abs)
            rr = self.big()
            V_.reciprocal_approx_fast(out=rr[:, lo:W], in_=ab[:, lo:W])
            ts_ = self.big()
            self.tt(V_, ts_[:, lo:W], dt_[:, lo:W], rr[:, lo:W], Alu.mult)
            nc.gpsimd.memset(ts_[:, 0:lo], 0.0)
            self.rel(dt_, ab, rr)
            if row_idx is not None:
                self.store_row(row_idx, ts_)
            return ts_

        T3s = tdiff(TEMA3, 6, 8)
        T1s = tdiff(TEMA3, 1, 6)
        T2s = tdiff(TEMAP2, 6, 7)
        self.rel(TEMAP2, TEMA3)

        # --- JX family ([LO, W): J1's validity bounds it) ---
        JXb = self.big()
        u = self.big()
        self.tt(V_, u[:, LO:W], J3[:, LO:W], T1s[:, LO:W], Alu.mult)
        v = self.big()
        self.tt(G_, v[:, LO:W], J1[:, LO:W], J2[:, LO:W], Alu.add)
        self.tt(G_, JXb[:, LO:W], u[:, LO:W], v[:, LO:W], Alu.add)
        self.rel(u, v, J3, T1s)
        F1 = self.big()
        self.tt(G_, F1[:, LO:W], J2[:, LO:W], T3s[:, LO:W], Alu.mult)
        self.rel(J2, T3s)
        F2 = self.big()
        self.tt(V_, F2[:, LO:W], J1[:, LO:W], T2s[:, LO:W], Alu.mult)
        self.rel(J1, T2s)
        self.store_row(19, JXb)
        self.store_row(20, F1)
        self.store_row(21, F2)

        EMA_JX = self.ema(JXb, 14, carry_lo=254)
        EMA_F1 = self.ema(F1, 15, carry_lo=254)
        EMA_F2 = self.ema(F2, 16, carry_lo=254)
        E8JX = self.ema(JXb, 17, raw=True, carry_lo=254)
        E8F1 = self.ema(F1, 18, raw=True, carry_lo=254)
        E8F2 = self.ema(F2, 19, raw=True, carry_lo=254)
        self.store_row(22, EMA_JX)
        self.store_row(23, EMA_F1)
        self.store_row(24, EMA_F2)

        def jx_combine(base, f1, f2, row_idx, lo=254):
            """out = base - 50 + 6*(f1 + f2)"""
            w_ = self.big()
            self.tt(G_, w_[:, lo:W], f1[:, lo:W], f2[:, lo:W], Alu.add)
            t1 = self.big()
            A_.activation(t1[:, lo:W], w_[:, lo:W], Act.Identity,
                          bias=self.m50[:, 0:1], scale=6.0)
            out = self.big()
            self.tt(G_, out[:, lo:W], t1[:, lo:W], base[:, lo:W], Alu.add)
            self.rel(w_, t1)
            self.store_row(row_idx, out)
            return out

        self.m50 = self.spool.tile([P, 1], F32, tag="c_m50")
        nc.gpsimd.memset(self.m50[:, :], -50.0)

        # need col 255 for the cross lag -> compute from col 254
        JX = jx_combine(JXb, F1, F2, 27, lo=254)
        EMAJX = jx_combine(EMA_JX, EMA_F1, EMA_F2, 28, lo=254)
        # EMAJX8 from raw scan tiles: a17*s17 - 50 + 6*(a18*s18 + a19*s19)
        a17, a18, a19 = self.alphas[17], self.alphas[18], self.alphas[19]
        m1 = self.big()
        A_.mul(m1[:, 254:W], E8F2[:, 254:W], a19 / a18)
        w_ = self.big()
        self.tt(G_, w_[:, 254:W], m1[:, 254:W], E8F1[:, 254:W], Alu.add)
        t2 = self.big()
        A_.mul(t2[:, 254:W], w_[:, 254:W], 6.0 * a18 / a17)
        z = self.big()
        self.tt(G_, z[:, 254:W], t2[:, 254:W], E8JX[:, 254:W], Alu.add)
        out8 = self.big()
        self.A.activation(out8[:, 254:W], z[:, 254:W], Act.Identity,
                          bias=self.m50[:, 0:1], scale=a17)
        self.rel(m1, w_, t2, z)
        self.store_row(29, out8)
        self.rel(JXb, F1, F2, EMA_JX, EMA_F1, EMA_F2, E8JX, E8F1, E8F2, out8)

        # --- crosses + segmented MA scans (Aprefix fix-up, no re-scan).
        # Exact cross conditions from two compares + lagged APs:
        #   up:  (JX>EJ) & (JXp<=EJp)  = gt[t] * (1-gt[t-1])
        #   dn:  (JX<EJ) & (JXp>=EJp)  = (1-ge[t]) * ge[t-1]
        gt = self.big()
        self.tt(V_, gt[:, 254:W], JX[:, 254:W], EMAJX[:, 254:W], Alu.is_gt)
        ge = self.big()
        self.tt(V_, ge[:, 254:W], JX[:, 254:W], EMAJX[:, 254:W], Alu.is_ge)
        ngt = self.big()
        A_.activation(ngt[:, 254:W], gt[:, 254:W], Act.Identity,
                      bias=self.one1[:, 0:1], scale=-1.0)
        nge = self.big()
        A_.activation(nge[:, 254:W], ge[:, 254:W], Act.Identity,
                      bias=self.one1[:, 0:1], scale=-1.0)

        def ma_side(updown, row_idx, diag_idx):
            cond = self.big()
            if updown == "up":
                self.tt(G_, cond[:, 255:W], gt[:, 255:W],
                        ngt[:, 254 : W - 1], Alu.mult)
            else:
                self.tt(G_, cond[:, 255:W], nge[:, 255:W],
                        ge[:, 254 : W - 1], Alu.mult)
            m_ = self.big()
            A_.activation(m_[:, 255:W], cond[:, 255:W], Act.Identity,
                          bias=self.one1[:, 0:1], scale=-1.0)
            dmask = self.big()
            self.tt(G_, dmask[:, C0:W], Ct[:, C0:W], m_[:, C0:W], Alu.mult)

            # local segmented scans (init 0)
            cnt_s = self.big()
            V_.tensor_tensor_scan(
                out=cnt_s[:, C0:W], data0=m_[:, C0:W], data1=m_[:, C0:W],
                initial=0.0, op0=Alu.mult, op1=Alu.add,
            )
            Ssum = self.big()
            V_.tensor_tensor_scan(
                out=Ssum[:, C0:W], data0=m_[:, C0:W], data1=dmask[:, C0:W],
                initial=0.0, op0=Alu.mult, op1=Alu.add,
            )
            self.rel(cond, dmask)

            # Aprefix: 1 while no event yet in this partition's chunk
            Apre = self.big()
            self.tt(V_, Apre[:, C0:W], cnt_s[:, C0:W], self.TGL[:, 0:CH],
                    Alu.is_equal)

            # per-partition initials via affine row chain:
            # A[p] = (no event in chunk p) ; E = local end value
            acol = self.small()
            V_.tensor_single_scalar(
                out=acol[:, 0:1], in_=cnt_s[:, W - 1 : W], scalar=float(CH),
                op=Alu.is_ge,
            )
            par = self.psrow.tile([1, P], F32, tag="psrow")
            self.mm(par[0:1, 0:P], acol[:, 0:1], self.Ident[:, :])
            arow = self.row()
            V_.tensor_copy(arow[0:1, 0:P], par[0:1, 0:P])
            self.rels(acol)

            def initcol(scan1_tile):
                """init[p] = chain state entering partition p (PSUM col)."""
                rowt = self.row()
                nc.gpsimd.memset(rowt[0:1, 0:1], 0.0)
                pbr = self.psrow.tile([1, P], F32, tag="psrow")
                self.mm(pbr[0:1, 0:P], scan1_tile[:, W - 1 : W], self.Ident[:, :])
                V_.tensor_tensor_scan(
                    out=rowt[0:1, 1 : P + 1], data0=arow[0:1, 0:P],
                    data1=pbr[0:1, 0:P], initial=0.0, op0=Alu.mult, op1=Alu.add,
                )
                pcc = self.pscol.tile([P, 1], F32, tag="pscol")
                self.mm(pcc[:, 0:1], rowt[0:1, 0:P], self.ones11[0:1, 0:1])
                self.relr(rowt)
                return pcc

            icnt = initcol(cnt_s)
            # corrected = local + Aprefix * init[p]
            V_.scalar_tensor_tensor(
                out=cnt_s[:, C0:W], in0=Apre[:, C0:W], scalar=icnt[:, 0:1],
                in1=cnt_s[:, C0:W], op0=Alu.mult, op1=Alu.add,
            )
            isum = initcol(Ssum)
            V_.scalar_tensor_tensor(
                out=Ssum[:, C0:W], in0=Apre[:, C0:W], scalar=isum[:, 0:1],
                in1=Ssum[:, C0:W], op0=Alu.mult, op1=Alu.add,
            )
            self.rel(Apre)
            self.relr(arow)

            # seen[t] = (cnt[t] <= ext_index[t]); monotone per partition, so
            # the first chunk col alone decides the diag
            seen = self.big()
            V_.scalar_tensor_tensor(
                out=seen[:, C0:W], in0=cnt_s[:, C0:W], scalar=1.0,
                in1=self.TG[:, 0:CH], op0=Alu.mult, op1=Alu.is_le,
            )

            # ma = (S * recip(max(cnt,1))) * seen
            rc = self.big()
            V_.tensor_scalar_max(rc[:, C0:W], cnt_s[:, C0:W], 1.0)
            rcp = self.big()
            V_.reciprocal_approx_fast(out=rcp[:, C0:W], in_=rc[:, C0:W])
            ma0 = self.big()
            self.tt(G_, ma0[:, C0:W], Ssum[:, C0:W], rcp[:, C0:W], Alu.mult)
            ma = self.big()
            self.tt(V_, ma[:, C0:W], ma0[:, C0:W], seen[:, C0:W], Alu.mult)
            self.rel(rc, rcp, ma0, cnt_s, Ssum)
            self.store_row(row_idx, ma)

            # diag: min over partitions 1..127 of seen[:, C0]
            drow = self.row()
            nc.sync.dma_start(out=drow[0:1, 0 : P - 1], in_=seen[1:P, C0 : C0 + 1])
            done = self.spool.tile([1, 1], F32, tag=f"diag{diag_idx}")
            V_.tensor_reduce(
                out=done[0:1, 0:1], in_=drow[0:1, 0 : P - 1],
                axis=mybir.AxisListType.X, op=Alu.min,
            )
            self.relr(drow)
            nc.sync.dma_start(
                out=self.DIAG[diag_idx : diag_idx + 1].rearrange(
                    "(a b) -> a b", a=1, b=1
                ),
                in_=done[0:1, 0:1],
            )
            self.rel(seen, ma)

        ma_side("dn", 25, 1)
        ma_side("up", 26, 0)

        self.rel(Ct, JX, EMAJX, gt, ge, ngt, nge)


_CACHE = {}


def _build(alphas, anchor):
    key = (tuple(round(float(a), 12) for a in alphas), round(float(anchor), 6))
    if key not in _CACHE:
        kb = KB(alphas, anchor)
        _CACHE[key] = kb.build()
    return _CACHE[key]


def _shard(x):
    """per-core input arrays [DLEN], clamp-padded on the global left."""
    outs = []
    for mcore in range(NCORES):
        lo = (mcore + 1) * S - DLEN
        if lo < 0:
            d = np.concatenate(
                [np.full(-lo, x[0], np.float32), x[0 : (mcore + 1) * S]]
            )
        else:
            d = x[lo : (mcore + 1) * S]
        outs.append(np.ascontiguousarray(d, np.float32))
    return outs


def _host_ma(C, JX, EJ):
    """exact host fallback for ma rows (numpy, global)."""
    f32 = np.float32
    T_ = len(C)
    lag = lambda x: np.concatenate([x[:1], x[:-1]])
    JXp, EJp = lag(JX), lag(EJ)
    res = {}
    cs = np.concatenate([[0.0], np.cumsum(C.astype(np.float64))])
    t_idx = np.arange(T_)
    for key, cond in (
        ("dn", (JX < EJ) & (JXp >= EJp)),
        ("up", (JX > EJ) & (JXp <= EJp)),
    ):
        last = np.maximum.accumulate(np.where(cond, t_idx, -1))
        csl = cs[np.maximum(last, 0) + 1]
        s = cs[t_idx + 1] - csl
        n = t_idx - last
        res[key] = np.where(
            (last >= 0) & (n > 0), s / np.maximum(n, 1), 0.0
        ).astype(f32)
    return res["dn"], res["up"]


def run_cores(inputs, trace=False):
    """compile (cached) + run on 8 cores; returns BassKernelResults."""
    C = np.ascontiguousarray(inputs["C"], np.float32)
    H = np.ascontiguousarray(inputs["H"], np.float32)
    L = np.ascontiguousarray(inputs["L"], np.float32)
    w = np.asarray(inputs["w_alphas"], np.float32)
    alphas = [float(1.0 / (1.0 + math.exp(-float(x)))) for x in w]
    nc = _build(alphas, float(C[0]))
    dc, dh, dl = _shard(C), _shard(H), _shard(L)
    in_maps = [
        {"DC": dc[m], "DH": dh[m], "DL": dl[m]} for m in range(NCORES)
    ]
    res = run_bass_kernel_spmd(
        nc, in_maps, core_ids=list(range(NCORES)), trace=trace
    )
    return res


def kernel(C, H, L, w_alphas):
    inputs = {"C": C, "H": H, "L": L, "w_alphas": w_alphas}
    res = run_cores(inputs)
    outs = [res.results[m]["OUT"].reshape(NROWS, EXT)[:, HALO:] for m in range(NCORES)]
    full = np.concatenate(outs, axis=1)
    full[0] = np.asarray(C, np.float32)
    full[1] = np.asarray(H, np.float32)
    full[2] = np.asarray(L, np.float32)

    # host patch: reference's partial-window std for the first 17 bars
    Cg = np.asarray(C, np.float64)[:17]
    for t in range(17):
        wdw = Cg[: t + 1]
        dis = math.sqrt(max(np.mean(wdw * wdw) - np.mean(wdw) ** 2, 0.0))
        full[3, t] = np.float32(full[4, t] + dis)
        full[5, t] = np.float32(full[4, t] - dis)

    # diag check: cross gap exceeded the halo on some core -> exact host fix
    need_fix = False
    for mcore in range(1, NCORES):
        dg = res.results[mcore]["DIAG"]
        if dg.min() < 0.5:
            need_fix = True
    if need_fix:
        ma_dn, ma_up = _host_ma(
            np.asarray(C, np.float32), full[27], full[28]
        )
        full[25] = ma_dn
        full[26] = ma_up
    return full.astype(np.float32)


# revision 20
# speedup vs baseline: 1.2255x; 1.2255x over previous
"""Trainium2 Bass kernel for nn_DifferentiableFeatureExtractor.

Strategy (8 NeuronCores, shard T=1048576 along time):
  - per-core extended domain EXT = S + 2048 halo = 133120 = 128 partitions x 1040
  - each partition holds a contiguous 1040-bar chunk plus a 256-bar AP halo
    (tile [128, 1296]); host supplies a 256-bar lead-in so partition 0's halo
    is real data (clamp-padded at the global left edge like the reference)
  - 20 EMAs as untruncated fp32 recurrences y = a*scale*s with s from
    tensor_tensor_scan (2-pass blocked scan; partition carry via a tiny PE
    shift-matmul read at col W-9 so scan2 covers [248, W) and no separate
    halo fill is needed).  The truncation correction (= (1-a)^K ~ 1e-3
    relative, smooth) is dropped - far inside the 2e-2 tolerance.
  - work is split across the three elementwise engines: DVE (vector),
    Pool (gpsimd; binary ops emulated via scalar_tensor_tensor for the
    0.6-efficiency path; never touches PSUM), and ACT (scalar) for all
    unary scale/abs/sqrt/copy work.
  - sliding max/min via log-doubling with shifted APs (halo-local)
  - rolling std via anchored windowed sums (doubling)
  - BARSLAST/MA_DYNAMIC as segmented scans (reset at cross events) with
    affine partition-carry fix-up (Aprefix trick: corrected = local +
    prefix-indicator * per-partition-initial); exact whenever the previous
    cross lies within the 2048-bar halo (diag output flags violations ->
    host fallback)
All heavy compute runs on device; host only shards, gathers, patches the
17 reference partial-window std bars, and handles the (never-observed)
diag fallback.
"""
import math

import numpy as np

import concourse.bacc as bacc
from concourse.bass_types import AP as BassAP
import concourse.mybir as mybir
from concourse import tile as ctile
from concourse.bass_utils import run_bass_kernel_spmd

F32 = mybir.dt.float32
Alu = mybir.AluOpType
Act = mybir.ActivationFunctionType

T = 1048576
NCORES = 8
S = T // NCORES            # 131072
P = 128
CH = 1040                  # chunk cols per partition
HP = 256                   # per-partition halo cols
W = HP + CH                # 1296
EXT = P * CH               # 133120
HALO = EXT - S             # 2048
DLEN = HP + EXT            # 133376
C0 = HP                    # chunk start col
LO = 248                   # extended-chunk start (8-col margin for lags)
NROWS = 30

# static truncation lengths (depend only on the reference's constant ALPHAS)
KS = [72, 72, 72, 286, 286, 286, 559, 89, 54, 47, 40, 30, 130, 30,
      30, 30, 30, 37, 37, 37]


class KB:
    """kernel builder with a simple big-tile freelist"""

    def __init__(self, alphas, anchor):
        self.alphas = [float(a) for a in alphas]
        self.anchor = float(anchor)
        nc = bacc.Bacc(None, target_bir_lowering=False)
        self.nc = nc
        self.DC = nc.dram_tensor("DC", [DLEN], F32, kind="ExternalInput")
        self.DH = nc.dram_tensor("DH", [DLEN], F32, kind="ExternalInput")
        self.DL = nc.dram_tensor("DL", [DLEN], F32, kind="ExternalInput")
        self.OUT = nc.dram_tensor("OUT", [NROWS * EXT], F32, kind="ExternalOutput")
        self.DIAG = nc.dram_tensor("DIAG", [2], F32, kind="ExternalOutput")
        self.free_big = []
        self.n_big = 0
        self.free_small = []
        self.n_small = 0
        self.free_row = []
        self.n_row = 0

    # ---- tile management ----
    def big(self):
        if self.free_big:
            return self.free_big.pop(0)
        t = self.pool.tile([P, W], F32, tag=f"big{self.n_big}")
        self.n_big += 1
        return t

    def rel(self, *ts):
        for t in ts:
            self.free_big.append(t)

    def small(self):
        if self.free_small:
            return self.free_small.pop()
        t = self.spool.tile([P, 1], F32, tag=f"small{self.n_small}")
        self.n_small += 1
        return t

    def rels(self, *ts):
        for t in ts:
            self.free_small.append(t)

    def row(self):
        if self.free_row:
            return self.free_row.pop()
        t = self.spool.tile([1, P + 1], F32, tag=f"row{self.n_row}")
        self.n_row += 1
        return t

    def relr(self, *ts):
        for t in ts:
            self.free_row.append(t)

    # ---- engine helpers ----
    def tt(self, eng, out, a, b, op):
        """binary tensor op (Pool supports only add/subtract/mult)."""
        if eng is self.G:
            assert op in (Alu.add, Alu.subtract, Alu.mult), op
        eng.tensor_tensor(out=out, in0=a, in1=b, op=op)

    # ---- IO ----
    def load_series(self, dram, eng="sync"):
        nc = self.nc
        t = self.big()
        base = dram[0:DLEN].rearrange("(a b) -> a b", a=1, b=DLEN)
        src_ap = BassAP(base.tensor, 0, [[CH, P], [1, W]])
        getattr(nc, eng).dma_start(out=t[:, 0:W], in_=src_ap)
        return t

    def store_row(self, r, t):
        nc = self.nc
        nc.sync.dma_start(
            out=self.OUT[r * EXT : (r + 1) * EXT].rearrange(
                "(p w) -> p w", p=P, w=CH
            ),
            in_=t[:, C0:W],
        )

    # ---- building blocks ----
    def ema(self, xt, i, scale=1.0, raw=False, xlo=0, carry_lo=None):
        """EMA of xt -> new tile.
        carry_lo=None: ONE scan over [xlo, W) warming up through the halo
          (requires (1-a)^HP tiny AND xt valid on [xlo, W)).
        carry_lo=lo: 2-pass blocked scan with PE carry; output valid [lo, W)
          (for slow alphas or inputs only valid from ~lo).
        Output y = a*scale*s via ACT unless raw."""
        nc = self.nc
        a = self.alphas[i]
        K = KS[i]
        c = 1.0 - a
        s = self.big()
        if carry_lo is None:
            assert c ** HP < 1e-5
            cbc = self.CONSTS[:, i : i + 1].broadcast_to([P, W - xlo])
            self.V.tensor_tensor_scan(
                out=s[:, xlo:W], data0=cbc,
                data1=xt[:, xlo:W], initial=0.0, op0=Alu.mult, op1=Alu.add,
            )
            lo = xlo
        else:
            lo = carry_lo
            V = min(CH, 2 * K + 16)
            cbc = self.CONSTS[:, i : i + 1].broadcast_to([P, V])
            self.V.tensor_tensor_scan(
                out=s[:, W - V : W], data0=cbc,
                data1=xt[:, W - V : W], initial=0.0, op0=Alu.mult, op1=Alu.add,
            )
            # carry[p] = s[p-1, ccol] (+ c^CH * s[p-2, ccol] for slow alphas)
            # enters one bar before col lo; chain step is CH bars either way.
            ccol = W - (C0 - lo) - 1
            mmat = self.M2.get(i, self.Sh1)
            pcar = self.pscol.tile([P, 1], F32, tag="pscol")
            self.mm(pcar[:, 0:1], mmat[:, :], s[:, ccol : ccol + 1])
            cbc2 = self.CONSTS[:, i : i + 1].broadcast_to([P, W - lo])
            self.V.tensor_tensor_scan(
                out=s[:, lo:W], data0=cbc2,
                data1=xt[:, lo:W], initial=pcar[:, 0:1], op0=Alu.mult, op1=Alu.add,
            )
        if raw:
            return s
        y = self.big()
        self.A.mul(y[:, lo:W], s[:, lo:W], a * scale)
        self.rel(s)
        return y

    def ema_corr(self, xt, i, scale=1.0):
        """EMA with the truncation correction (TEMA chains, i in 0..5):
        y = a*scale*(s[t] - c^K s[t-K]) on [0, W); head cols [0, K) read
        s[t-K] from the previous partition's tail via a PE shift (partition
        0 gets zero-history - it lies in the discarded global halo)."""
        nc = self.nc
        a = self.alphas[i]
        K = KS[i]
        c = 1.0 - a
        cK = float(c) ** K
        if c ** HP < 1e-5:
            s = self.ema(xt, i, raw=True)
        else:
            s = self.ema(xt, i, raw=True, carry_lo=0)
        q = self.big()
        self.V.scalar_tensor_tensor(
            out=q[:, K:W], in0=s[:, 0 : W - K], scalar=cK,
            in1=s[:, K:W], op0=Alu.mult, op1=Alu.subtract,
        )
        ph = self.pshalo.tile([P, 286], F32, tag="psh")
        self.mm(ph[:, 0:K], self.Sh1[:, :], s[:, CH - K : CH])
        self.V.scalar_tensor_tensor(
            out=q[:, 0:K], in0=ph[:, 0:K], scalar=cK,
            in1=s[:, 0:K], op0=Alu.mult, op1=Alu.subtract,
        )
        self.rel(s)
        y = self.big()
        self.A.mul(y[:, 0:W], q[:, 0:W], -a * scale)
        self.rel(q)
        return y

    def winchain(self, xt, jmax, op, eng):
        """doubling chain for sliding max/min, valid on ALL of [0, W) with
        partial windows at the left edge (dst[:, :sh] copied from cur)."""
        chain = {1: xt}
        cur = xt
        for j in range(jmax):
            sh = 1 << j
            dst = self.big()
            self.tt(
                eng, dst[:, sh:W], cur[:, sh:W], cur[:, 0 : W - sh], op,
            )
            self.A.copy(dst[:, 0:sh], cur[:, 0:sh])
            cur = dst
            chain[sh * 2] = dst
        return chain

    def wincombine(self, chain, n, op, eng, lo=0):
        """window-n result from a doubling chain (partial windows left of
        col n-1, matching the reference's edge behaviour inside the halo)."""
        J = 1 << int(math.floor(math.log2(n)))
        r = n - J
        cur = chain[J]
        out = self.big()
        if r > 0:
            self.tt(
                eng, out[:, max(r, lo):W], cur[:, max(r, lo):W],
                cur[:, max(r, lo) - r : W - r], op,
            )
            if lo < r:
                self.A.copy(out[:, lo:r], cur[:, lo:r])
        else:
            eng.tensor_copy(out[:, 0:W], cur[:, 0:W])
        return out

    def winsum18(self, xt, xlo, eng):
        """rolling 18-window sum of xt (valid from xlo); returns tile valid
        [xlo+31, W)."""
        tiles = []
        cur = xt
        curlo = xlo
        chain1 = None
        for j in range(4):
            sh = 1 << j
            dst = self.big()
            tiles.append(dst)
            self.tt(
                eng, dst[:, curlo + sh : W], cur[:, curlo + sh : W],
                cur[:, curlo : W - sh], Alu.add,
            )
            cur = dst
            curlo += sh
            if j == 0:
                chain1 = dst  # window-2 sums
        out = self.big()
        # S18[t] = W16[t] + W2[t-16]
        lo = curlo + 16
        self.tt(eng, out[:, lo:W], cur[:, lo:W], chain1[:, lo - 16 : W - 16], Alu.add)
        for t in tiles:
            self.rel(t)
        return out, lo

    # ---- full pipeline ----
    def build(self):
        nc = self.nc
        with ctile.TileContext(nc) as tc:
            with tc.tile_pool(name="big", bufs=1) as pool, tc.tile_pool(
                name="small", bufs=1
            ) as spool, tc.tile_pool(name="psc", bufs=2, space="PSUM") as pscol, \
                 tc.tile_pool(name="psh", bufs=2, space="PSUM") as pshalo, \
                 tc.tile_pool(name="psr", bufs=2, space="PSUM") as psrow:
                self.pool = pool
                self.spool = spool
                self.pscol = pscol
                self.pshalo = pshalo
                self.psrow = psrow
                self.emit()
        nc.finalize()
        return nc

    def mm(self, out, lhsT, rhs):
        self.nc.tensor.matmul(out, lhsT, rhs, start=True, stop=True)

    def emit(self):
        nc = self.nc
        I32 = mybir.dt.int32
        self.V = nc.vector
        self.G = nc.gpsimd
        self.A = nc.scalar
        V_, G_, A_ = self.V, self.G, self.A

        self.one1 = self.spool.tile([P, 1], F32, tag="c_one1")
        nc.gpsimd.memset(self.one1[:, :], 1.0)
        self.nanch = self.spool.tile([P, 1], F32, tag="c_nanch")
        nc.gpsimd.memset(self.nanch[:, :], -self.anchor)
        # PE helper constants: ii[p, m] = m - p; Sh1 = (ii==1); Ident = (ii==0)
        ii = self.spool.tile([P, P], I32, tag="c_iota")
        nc.gpsimd.iota(ii[:, :], pattern=[[1, P]], base=0, channel_multiplier=-1)
        self.Sh1 = self.spool.tile([P, P], F32, tag="c_sh1")
        nc.vector.tensor_single_scalar(
            out=self.Sh1[:, :], in_=ii[:, :], scalar=1, op=Alu.is_equal
        )
        self.Ident = self.spool.tile([P, P], F32, tag="c_ident")
        nc.vector.tensor_single_scalar(
            out=self.Ident[:, :], in_=ii[:, :], scalar=0, op=Alu.is_equal
        )
        # second-order carry matrices for slow alphas (cF > 1e-10)
        self.M2 = {}
        sh2 = None
        for i, a in enumerate(self.alphas):
            cF = (1.0 - a) ** CH
            if cF > 1e-10:
                if sh2 is None:
                    sh2 = self.spool.tile([P, P], F32, tag="c_sh2")
                    nc.vector.tensor_single_scalar(
                        out=sh2[:, :], in_=ii[:, :], scalar=2, op=Alu.is_equal
                    )
                m = self.spool.tile([P, P], F32, tag=f"c_m2_{i}")
                nc.vector.scalar_tensor_tensor(
                    out=m[:, :], in0=sh2[:, :], scalar=float(cF), in1=self.Sh1[:, :],
                    op0=Alu.mult, op1=Alu.add,
                )
                self.M2[i] = m
        # per-conv scan multiplier constants: CONSTS[:, i] = 1 - alpha_i
        self.CONSTS = self.spool.tile([P, len(self.alphas)], F32, tag="c_scanmul")
        for i, a in enumerate(self.alphas):
            nc.gpsimd.memset(self.CONSTS[:, i : i + 1], 1.0 - a)
        self.ones11 = self.spool.tile([1, 1], F32, tag="c_ones11")
        nc.gpsimd.memset(self.ones11[:, :], 1.0)
        # TG: global ext index per bar (for `seen`); TGL: local index + 1
        self.TG = self.spool.tile([P, CH], I32, tag="c_tg")
        nc.gpsimd.iota(self.TG[:, :], pattern=[[1, CH]], base=0, channel_multiplier=CH)
        self.TGL = self.spool.tile([P, CH], I32, tag="c_tgl")
        nc.gpsimd.iota(self.TGL[:, :], pattern=[[1, CH]], base=1, channel_multiplier=0)

        Ht = self.load_series(self.DH)
        Lt = self.load_series(self.DL, eng="gpsimd")
        Ct = self.load_series(self.DC)

        # --- sliding-window chains: H on DVE, L on Pool ---
        hchain = self.winchain(Ht, 7, Alu.max, V_)
        lchain = self.winchain(Lt, 7, Alu.min, V_)

        def rsvf(nw, lo=0):
            """rsv in [0,1] on [lo, W) (left edge = partial windows)."""
            hh = self.wincombine(hchain, nw, Alu.max, V_, lo=lo)
            ll = self.wincombine(lchain, nw, Alu.min, V_, lo=lo)
            hl = self.big()
            self.tt(V_, hl[:, lo:W], hh[:, lo:W], ll[:, lo:W], Alu.subtract)
            rcp = self.big()
            V_.reciprocal_approx_fast(out=rcp[:, lo:W], in_=hl[:, lo:W])
            num = self.big()
            self.tt(V_, num[:, lo:W], Ct[:, lo:W], ll[:, lo:W], Alu.subtract)
            r0 = self.big()
            self.tt(V_, r0[:, lo:W], num[:, lo:W], rcp[:, lo:W], Alu.mult)
            self.rel(hh, ll, hl, rcp, num)
            rsv01 = self.big()
            V_.tensor_scalar(
                out=rsv01[:, lo:W], in0=r0[:, lo:W], scalar1=0.0, scalar2=1.0,
                op0=Alu.max, op1=Alu.min,
            )
            self.rel(r0)
            return rsv01

        def jof(Kv, Dv, rows, lo=0):
            k3 = self.big()
            A_.mul(k3[:, lo:W], Kv[:, lo:W], 3.0)
            d2 = self.big()
            A_.mul(d2[:, lo:W], Dv[:, lo:W], 2.0)
            Jv = self.big()
            self.tt(V_, Jv[:, lo:W], k3[:, lo:W], d2[:, lo:W], Alu.subtract)
            self.rel(k3, d2)
            for ridx, tt_ in zip(rows, (Kv, Dv, Jv)):
                if ridx is not None:
                    self.store_row(ridx, tt_)
            return Jv

        rsv1 = rsvf(204, lo=LO)
        rsv2 = rsvf(18)
        K1 = self.ema(rsv1, 6, 100.0, carry_lo=LO)
        K2 = self.ema(rsv2, 8, 100.0)
        self.rel(rsv1, rsv2)
        D1 = self.ema(K1, 7, carry_lo=LO)
        D2 = self.ema(K2, 9)
        J1 = jof(K1, D1, (9, 10, 11), lo=LO)
        J2 = jof(K2, D2, (12, 13, 14), lo=LO)
        self.rel(K1, D1, K2, D2)

        # --- TEMA3 + TEMAP2 chains (engine-split pairs) ---
        # EMA1/EMA2 carry a folded 3x; EMA3 unfolds it (ema is linear), so
        # TEMA3 = EMA1x3 - EMA2x3 + EMA3 with Pool-only combines.  The TEMA
        # chains use truncation-corrected EMAs (the T-ratio rows are the
        # precision-critical consumers).
        EMA1 = self.ema_corr(Ct, 0, 3.0)
        E21 = self.ema_corr(Ct, 3, 3.0)
        EMA2 = self.ema_corr(EMA1, 1)
        E221 = self.ema_corr(E21, 4)
        EMA3 = self.ema_corr(EMA2, 2, 1.0 / 3.0)
        E231 = self.ema_corr(E221, 5, 1.0 / 3.0)
        TEMA3 = self.big()
        d = self.big()
        self.tt(V_, d[:, 236:W], EMA1[:, 236:W], EMA2[:, 236:W], Alu.subtract)
        self.tt(V_, TEMA3[:, 236:W], d[:, 236:W], EMA3[:, 236:W], Alu.add)
        self.rel(EMA1, EMA2, EMA3, d)
        self.store_row(4, TEMA3)
        TEMAP2 = self.big()
        d = self.big()
        self.tt(V_, d[:, 236:W], E21[:, 236:W], E221[:, 236:W], Alu.subtract)
        self.tt(V_, TEMAP2[:, 236:W], d[:, 236:W], E231[:, 236:W], Alu.add)
        self.rel(E21, E221, E231, d)

        # --- stdp(C,18) anchored at global C[0] ---
        dev = self.big()
        self.A.activation(dev[:, 225:W], Ct[:, 225:W], Act.Identity, bias=self.nanch[:, 0:1])
        dev2 = self.big()
        self.A.activation(dev2[:, 225:W], Ct[:, 225:W], Act.Square, bias=self.nanch[:, 0:1])
        S18, lo1 = self.winsum18(dev, 225, V_)
        Q18, lo2 = self.winsum18(dev2, 225, G_)
        self.rel(dev, dev2)
        m = self.big()
        self.A.mul(m[:, lo1:W], S18[:, lo1:W], 1.0 / 18.0)
        ex2 = self.big()
        self.A.mul(ex2[:, lo2:W], Q18[:, lo2:W], 1.0 / 18.0)
        self.rel(S18, Q18)
        mm_ = self.big()
        self.A.square(mm_[:, lo1:W], m[:, lo1:W])
        var = self.big()
        self.tt(G_, var[:, lo1:W], ex2[:, lo1:W], mm_[:, lo1:W], Alu.subtract)
        V_.tensor_scalar_max(var[:, lo1:W], var[:, lo1:W], 0.0)
        DIS = self.big()
        self.A.activation(DIS[:, lo1:W], var[:, lo1:W], Act.Sqrt)
        self.rel(m, ex2, mm_, var)
        TEU3 = self.big()
        self.tt(G_, TEU3[:, C0:W], TEMA3[:, C0:W], DIS[:, C0:W], Alu.add)
        TED = self.big()
        self.tt(G_, TED[:, C0:W], TEMA3[:, C0:W], DIS[:, C0:W], Alu.subtract)
        self.store_row(3, TEU3)
        self.store_row(5, TED)
        self.rel(DIS, TEU3, TED)

        rsv3 = rsvf(9)
        rsvn = rsvf(36)
        K3 = self.ema(rsv3, 10, 100.0)
        KN3 = self.ema(rsvn, 12, 100.0)
        self.rel(rsv3, rsvn)
        D3 = self.ema(K3, 11)
        DN3 = self.ema(KN3, 13)
        J3 = jof(K3, D3, (15, 16, 17), lo=LO)
        JN3 = jof(KN3, DN3, (None, None, 18), lo=LO)
        self.rel(K3, D3, KN3, DN3, JN3)
        for ch_ in (hchain, lchain):
            for kk, tt_ in ch_.items():
                if kk > 1:
                    self.rel(tt_)
        self.rel(Ht, Lt)

        # --- T ratios (no +eps: |r| ~ 100 so the 1e-8 is immaterial).
        # Computed from col `lag` so JXb/F1/F2 are near-full-width and the
        # JX EMAs can warm up through the halo with a single scan.
        def tdiff(xt, lag, row_idx, lo=242):
            dt_ = self.big()
            self.tt(
                V_, dt_[:, lo:W], xt[:, lo:W], xt[:, lo - lag : W - lag],
                Alu.subtract,
            )
            ab = self.big()
            self.A.activation(ab[:, lo:W], xt[:, lo - lag : W - lag], Act.Abs)
            rr = self.big()
            V_.reciprocal_approx_fast(out=rr[:, lo:W], in_=ab[:, lo:W])
            ts_ = self.big()
            self.tt(V_, ts_[:, lo:W], dt_[:, lo:W], rr[:, lo:W], Alu.mult)
            self.rel(dt_, ab, rr)
            if row_idx is not None:
                self.store_row(row_idx, ts_)
            return ts_

        T3s = tdiff(TEMA3, 6, 8)
        T1s = tdiff(TEMA3, 1, 6)
        T2s = tdiff(TEMAP2, 6, 7)
        self.rel(TEMAP2, TEMA3)

        # --- JX family ([LO, W): J1's validity bounds it) ---
        JXb = self.big()
        u = self.big()
        self.tt(V_, u[:, LO:W], J3[:, LO:W], T1s[:, LO:W], Alu.mult)
        v = self.big()
        self.tt(G_, v[:, LO:W], J1[:, LO:W], J2[:, LO:W], Alu.add)
        self.tt(V_, JXb[:, LO:W], u[:, LO:W], v[:, LO:W], Alu.add)
        self.rel(u, v, J3, T1s)
        F1 = self.big()
        self.tt(G_, F1[:, LO:W], J2[:, LO:W], T3s[:, LO:W], Alu.mult)
        self.rel(J2, T3s)
        F2 = self.big()
        self.tt(G_, F2[:, LO:W], J1[:, LO:W], T2s[:, LO:W], Alu.mult)
        self.rel(J1, T2s)
        self.store_row(19, JXb)
        self.store_row(20, F1)
        self.store_row(21, F2)

        EMA_JX = self.ema(JXb, 14, carry_lo=254)
        EMA_F1 = self.ema(F1, 15, carry_lo=254)
        EMA_F2 = self.ema(F2, 16, carry_lo=254)
        E8JX = self.ema(JXb, 17, raw=True, carry_lo=254)
        E8F1 = self.ema(F1, 18, raw=True, carry_lo=254)
        E8F2 = self.ema(F2, 19, raw=True, carry_lo=254)
        self.store_row(22, EMA_JX)
        self.store_row(23, EMA_F1)
        self.store_row(24, EMA_F2)

        def jx_combine(base, f1, f2, row_idx, lo=254):
            """out = base - 50 + 6*(f1 + f2)"""
            w_ = self.big()
            self.tt(G_, w_[:, lo:W], f1[:, lo:W], f2[:, lo:W], Alu.add)
            t1 = self.big()
            A_.activation(t1[:, lo:W], w_[:, lo:W], Act.Identity,
                          bias=self.m50[:, 0:1], scale=6.0)
            out = self.big()
            self.tt(V_, out[:, lo:W], t1[:, lo:W], base[:, lo:W], Alu.add)
            self.rel(w_, t1)
            self.store_row(row_idx, out)
            return out

        self.m50 = self.spool.tile([P, 1], F32, tag="c_m50")
        nc.gpsimd.memset(self.m50[:, :], -50.0)

        # need col 255 for the cross lag -> compute from col 254
        JX = jx_combine(JXb, F1, F2, 27, lo=254)
        EMAJX = jx_combine(EMA_JX, EMA_F1, EMA_F2, 28, lo=254)
        # EMAJX8 from raw scan tiles: a17*s17 - 50 + 6*(a18*s18 + a19*s19)
        a17, a18, a19 = self.alphas[17], self.alphas[18], self.alphas[19]
        m1 = self.big()
        A_.mul(m1[:, 254:W], E8F2[:, 254:W], a19 / a18)
        w_ = self.big()
        self.tt(G_, w_[:, 254:W], m1[:, 254:W], E8F1[:, 254:W], Alu.add)
        t2 = self.big()
        A_.mul(t2[:, 254:W], w_[:, 254:W], 6.0 * a18 / a17)
        z = self.big()
        self.tt(G_, z[:, 254:W], t2[:, 254:W], E8JX[:, 254:W], Alu.add)
        out8 = self.big()
        self.A.activation(out8[:, 254:W], z[:, 254:W], Act.Identity,
                          bias=self.m50[:, 0:1], scale=a17)
        self.rel(m1, w_, t2, z)
        self.store_row(29, out8)
        self.rel(JXb, F1, F2, EMA_JX, EMA_F1, EMA_F2, E8JX, E8F1, E8F2, out8)

        # --- crosses + segmented MA scans (Aprefix fix-up, no re-scan).
        # Exact cross conditions from two compares + lagged APs:
        #   up:  (JX>EJ) & (JXp<=EJp)  = gt[t] * (1-gt[t-1])
        #   dn:  (JX<EJ) & (JXp>=EJp)  = (1-ge[t]) * ge[t-1]
        gt = self.big()
        self.tt(V_, gt[:, 254:W], JX[:, 254:W], EMAJX[:, 254:W], Alu.is_gt)
        ge = self.big()
        self.tt(V_, ge[:, 254:W], JX[:, 254:W], EMAJX[:, 254:W], Alu.is_ge)
        ngt = self.big()
        A_.activation(ngt[:, 254:W], gt[:, 254:W], Act.Identity,
                      bias=self.one1[:, 0:1], scale=-1.0)
        nge = self.big()
        A_.activation(nge[:, 254:W], ge[:, 254:W], Act.Identity,
                      bias=self.one1[:, 0:1], scale=-1.0)

        def ma_side(updown, row_idx, diag_idx):
            cond = self.big()
            if updown == "up":
                self.tt(V_, cond[:, 255:W], gt[:, 255:W],
                        ngt[:, 254 : W - 1], Alu.mult)
            else:
                self.tt(V_, cond[:, 255:W], nge[:, 255:W],
                        ge[:, 254 : W - 1], Alu.mult)
            m_ = self.big()
            A_.activation(m_[:, 255:W], cond[:, 255:W], Act.Identity,
                          bias=self.one1[:, 0:1], scale=-1.0)
            dmask = self.big()
            self.tt(G_, dmask[:, C0:W], Ct[:, C0:W], m_[:, C0:W], Alu.mult)

            # local segmented scans (init 0)
            cnt_s = self.big()
            V_.tensor_tensor_scan(
                out=cnt_s[:, C0:W], data0=m_[:, C0:W], data1=m_[:, C0:W],
                initial=0.0, op0=Alu.mult, op1=Alu.add,
            )
            Ssum = self.big()
            V_.tensor_tensor_scan(
                out=Ssum[:, C0:W], data0=m_[:, C0:W], data1=dmask[:, C0:W],
                initial=0.0, op0=Alu.mult, op1=Alu.add,
            )
            self.rel(cond, dmask)

            # Aprefix: 1 while no event yet in this partition's chunk
            Apre = self.big()
            self.tt(V_, Apre[:, C0:W], cnt_s[:, C0:W], self.TGL[:, 0:CH],
                    Alu.is_equal)

            # per-partition initials via affine row chain:
            # A[p] = (no event in chunk p) ; E = local end value
            acol = self.small()
            V_.tensor_single_scalar(
                out=acol[:, 0:1], in_=cnt_s[:, W - 1 : W], scalar=float(CH),
                op=Alu.is_ge,
            )
            par = self.psrow.tile([1, P], F32, tag="psrow")
            self.mm(par[0:1, 0:P], acol[:, 0:1], self.Ident[:, :])
            arow = self.row()
            V_.tensor_copy(arow[0:1, 0:P], par[0:1, 0:P])
            self.rels(acol)

            def initcol(scan1_tile):
                """init[p] = chain state entering partition p (PSUM col)."""
                rowt = self.row()
                nc.gpsimd.memset(rowt[0:1, 0:1], 0.0)
                pbr = self.psrow.tile([1, P], F32, tag="psrow")
                self.mm(pbr[0:1, 0:P], scan1_tile[:, W - 1 : W], self.Ident[:, :])
                V_.tensor_tensor_scan(
                    out=rowt[0:1, 1 : P + 1], data0=arow[0:1, 0:P],
                    data1=pbr[0:1, 0:P], initial=0.0, op0=Alu.mult, op1=Alu.add,
                )
                pcc = self.pscol.tile([P, 1], F32, tag="pscol")
                self.mm(pcc[:, 0:1], rowt[0:1, 0:P], self.ones11[0:1, 0:1])
                self.relr(rowt)
                return pcc

            icnt = initcol(cnt_s)
            # corrected = local + Aprefix * init[p]
            V_.scalar_tensor_tensor(
                out=cnt_s[:, C0:W], in0=Apre[:, C0:W], scalar=icnt[:, 0:1],
                in1=cnt_s[:, C0:W], op0=Alu.mult, op1=Alu.add,
            )
            isum = initcol(Ssum)
            V_.scalar_tensor_tensor(
                out=Ssum[:, C0:W], in0=Apre[:, C0:W], scalar=isum[:, 0:1],
                in1=Ssum[:, C0:W], op0=Alu.mult, op1=Alu.add,
            )
            self.rel(Apre)
            self.relr(arow)

            # seen[t] = (cnt[t] <= ext_index[t]); monotone per partition, so
            # the first chunk col alone decides the diag
            seen = self.big()
            V_.scalar_tensor_tensor(
                out=seen[:, C0:W], in0=cnt_s[:, C0:W], scalar=1.0,
                in1=self.TG[:, 0:CH], op0=Alu.mult, op1=Alu.is_le,
            )

            # ma = (S * recip(max(cnt,1))) * seen
            rc = self.big()
            V_.tensor_scalar_max(rc[:, C0:W], cnt_s[:, C0:W], 1.0)
            rcp = self.big()
            V_.reciprocal_approx_fast(out=rcp[:, C0:W], in_=rc[:, C0:W])
            ma0 = self.big()
            self.tt(G_, ma0[:, C0:W], Ssum[:, C0:W], rcp[:, C0:W], Alu.mult)
            ma = self.big()
            self.tt(V_, ma[:, C0:W], ma0[:, C0:W], seen[:, C0:W], Alu.mult)
            self.rel(rc, rcp, ma0, cnt_s, Ssum)
            self.store_row(row_idx, ma)

            # diag: min over partitions 1..127 of seen[:, C0]
            drow = self.row()
            nc.sync.dma_start(out=drow[0:1, 0 : P - 1], in_=seen[1:P, C0 : C0 + 1])
            done = self.spool.tile([1, 1], F32, tag=f"diag{diag_idx}")
            V_.tensor_reduce(
                out=done[0:1, 0:1], in_=drow[0:1, 0 : P - 1],
                axis=mybir.AxisListType.X, op=Alu.min,
            )
            self.relr(drow)
            nc.sync.dma_start(
                out=self.DIAG[diag_idx : diag_idx + 1].rearrange(
                    "(a b) -> a b", a=1, b=1
                ),
                in_=done[0:1, 0:1],
            )
            self.rel(seen, ma)

        ma_side("dn", 25, 1)
        ma_side("up", 26, 0)

        self.rel(Ct, JX, EMAJX, gt, ge, ngt, nge)


_CACHE = {}


def _build(alphas, anchor):
    key = (tuple(round(float(a), 12) for a in alphas), round(float(anchor), 6))
    if key not in _CACHE:
        kb = KB(alphas, anchor)
        _CACHE[key] = kb.build()
    return _CACHE[key]


def _shard(x):
    """per-core input arrays [DLEN], clamp-padded on the global left."""
    outs = []
    for mcore in range(NCORES):
        lo = (mcore + 1) * S - DLEN
        if lo < 0:
            d = np.concatenate(
                [np.full(-lo, x[0], np.float32), x[0 : (mcore + 1) * S]]
            )
        else:
            d = x[lo : (mcore + 1) * S]
        outs.append(np.ascontiguousarray(d, np.float32))
    return outs


def _host_ma(C, JX, EJ):
    """exact host fallback for ma rows (numpy, global)."""
    f32 = np.float32
    T_ = len(C)
    lag = lambda x: np.concatenate([x[:1], x[:-1]])
    JXp, EJp = lag(JX), lag(EJ)
    res = {}
    cs = np.concatenate([[0.0], np.cumsum(C.astype(np.float64))])
    t_idx = np.arange(T_)
    for key, cond in (
        ("dn", (JX < EJ) & (JXp >= EJp)),
        ("up", (JX > EJ) & (JXp <= EJp)),
    ):
        last = np.maximum.accumulate(np.where(cond, t_idx, -1))
        csl = cs[np.maximum(last, 0) + 1]
        s = cs[t_idx + 1] - csl
        n = t_idx - last
        res[key] = np.where(
            (last >= 0) & (n > 0), s / np.maximum(n, 1), 0.0
        ).astype(f32)
    return res["dn"], res["up"]


def run_cores(inputs, trace=False):
    """compile (cached) + run on 8 cores; returns BassKernelResults."""
    C = np.ascontiguousarray(inputs["C"], np.float32)
    H = np.ascontiguousarray(inputs["H"], np.float32)
    L = np.ascontiguousarray(inputs["L"], np.float32)
    w = np.asarray(inputs["w_alphas"], np.float32)
    alphas = [float(1.0 / (1.0 + math.exp(-float(x)))) for x in w]
    nc = _build(alphas, float(C[0]))
    dc, dh, dl = _shard(C), _shard(H), _shard(L)
    in_maps = [
        {"DC": dc[m], "DH": dh[m], "DL": dl[m]} for m in range(NCORES)
    ]
    res = run_bass_kernel_spmd(
        nc, in_maps, core_ids=list(range(NCORES)), trace=trace
    )
    return res


def kernel(C, H, L, w_alphas):
    inputs = {"C": C, "H": H, "L": L, "w_alphas": w_alphas}
    res = run_cores(inputs)
    outs = [res.results[m]["OUT"].reshape(NROWS, EXT)[:, HALO:] for m in range(NCORES)]
    full = np.concatenate(outs, axis=1)
    full[0] = np.asarray(C, np.float32)
    full[1] = np.asarray(H, np.float32)
    full[2] = np.asarray(L, np.float32)

    # host patch: reference's partial-window std for the first 17 bars
    Cg = np.asarray(C, np.float64)[:17]
    for t in range(17):
        wdw = Cg[: t + 1]
        dis = math.sqrt(max(np.mean(wdw * wdw) - np.mean(wdw) ** 2, 0.0))
        full[3, t] = np.float32(full[4, t] + dis)
        full[5, t] = np.float32(full[4, t] - dis)

    # diag check: cross gap exceeded the halo on some core -> exact host fix
    need_fix = False
    for mcore in range(1, NCORES):
        dg = res.results[mcore]["DIAG"]
        if dg.min() < 0.5:
            need_fix = True
    if need_fix:
        ma_dn, ma_up = _host_ma(
            np.asarray(C, np.float32), full[27], full[28]
        )
        full[25] = ma_dn
        full[26] = ma_up
    return full.astype(np.float32)
